# revision 7
# baseline (speedup 1.0000x reference)
"""Bass/Trainium2 kernel for nn_HailNet_42975442763785 (GNN message passing).

Math insight: the COO adjacency built by the model only references node
indices in [0, 4111) (kj = k + j with k<16, j<4096; all other index vectors
stay below 4111), and the coalesced matrix A is banded with offsets in
[-80, 80].  Therefore h1 = xf @ A.T is zero outside its first 4111 columns
and the [48,65536] @ [65536,256] embedding matmul reduces exactly to
[48,4111] @ [4111,256].  Stage A (A @ xfT) becomes a block-tridiagonal
matmul over 128-wide blocks.

Sharding (8 cores):
  - The 4111 (padded to 5120 = 40*128) contraction axis is split 5 blocks
    per core.  Stage A needs only a 1-block halo of xfT -> no communication.
  - Stage B computes per-core partial t2 pre-activations [256, 48];
    one AllReduce combines them.
  - The tiny tail (lin1, GRU over T=12, final MLP) is computed redundantly
    on every core; core 0's output is returned.

Everything on-device runs in a feature-on-partition layout ("T layout"):
activations are [128, ...] tiles with the feature dim on partitions and
(batch*time) on the free dim, so all elementwise GRU ops are [128, <=16].
"""

from contextlib import ExitStack

import numpy as np

import concourse.bass as bass
import concourse.tile as tile
from concourse import bacc, mybir
from concourse.bass_utils import run_bass_kernel_spmd

F32 = mybir.dt.float32
AF = mybir.ActivationFunctionType
ALU = mybir.AluOpType

N_CORES = 8
BLK = 128
NB = 5                    # I-blocks per core
NBLOCKS = N_CORES * NB    # 40 padded blocks
SUP = 4111                # true support of the adjacency
N = 65536
BT, B, T = 48, 4, 12
EMB, HID, G3 = 256, 256, 768


# ---------------------------------------------------------------- device code

def build_program(repeat: int = 1, loads_in_body: bool = False):
    """loads_in_body=True re-DMAs all large inputs every repeat iteration so
    the repeat-slope timing includes the input-streaming cost of a one-shot
    run (this is a memory-regime problem)."""
    nc = bacc.Bacc("TRN2", target_bir_lowering=False, debug=False,
                   num_devices=N_CORES)

    # per-core sharded inputs
    at_d = nc.dram_tensor("at", [3 * NB, BLK, BLK], F32, kind="ExternalInput")
    xh_d = nc.dram_tensor("xh", [NB + 2, BLK, BT], F32, kind="ExternalInput")
    wes_d = nc.dram_tensor("wes", [NB, BLK, EMB], F32, kind="ExternalInput")
    # replicated weights
    wl1_d = nc.dram_tensor("wl1t", [2, BLK, EMB], F32, kind="ExternalInput")
    wih_d = nc.dram_tensor("wiht", [2, BLK, G3], F32, kind="ExternalInput")
    whh_d = nc.dram_tensor("whht", [2, BLK, G3], F32, kind="ExternalInput")
    wf0_d = nc.dram_tensor("wf0t", [2, BLK, 16], F32, kind="ExternalInput")
    wf1_d = nc.dram_tensor("wf1t", [16, 16], F32, kind="ExternalInput")
    wf2_d = nc.dram_tensor("wf2t", [16, 1], F32, kind="ExternalInput")
    bemb_d = nc.dram_tensor("bemb", [BLK, 2], F32, kind="ExternalInput")
    bl1_d = nc.dram_tensor("bl1", [BLK, 2], F32, kind="ExternalInput")
    bxp_d = nc.dram_tensor("bxp", [BLK, 6], F32, kind="ExternalInput")
    bhn_d = nc.dram_tensor("bhn", [BLK, 2], F32, kind="ExternalInput")
    h0_d = nc.dram_tensor("h0c", [BLK, 8], F32, kind="ExternalInput")
    bf0_d = nc.dram_tensor("bf0", [16, 1], F32, kind="ExternalInput")
    bf1_d = nc.dram_tensor("bf1", [16, 1], F32, kind="ExternalInput")
    bf2_d = nc.dram_tensor("bf2", [1, 1], F32, kind="ExternalInput")
    out_d = nc.dram_tensor("out", [1, B], F32, kind="ExternalOutput")

    with tile.TileContext(nc) as tc, ExitStack() as ctx:
        const = ctx.enter_context(tc.tile_pool(name="const", bufs=1))
        work = ctx.enter_context(tc.tile_pool(name="work", bufs=2))
        gru = ctx.enter_context(tc.tile_pool(name="gru", bufs=2))
        psAB = ctx.enter_context(tc.tile_pool(name="psAB", bufs=2, space="PSUM"))
        psX = ctx.enter_context(tc.tile_pool(name="psX", bufs=1, space="PSUM"))
        psG = ctx.enter_context(tc.tile_pool(name="psG", bufs=2, space="PSUM"))
        dram = ctx.enter_context(tc.tile_pool(name="dram", bufs=2, space="DRAM"))

        def emit_loads(pool):
            """DMA the large per-core inputs + weights into SBUF tiles."""
            at_sb = pool.tile([BLK, 3 * NB, BLK], F32, tag="at_sb")
            for j in range(3 * NB):
                nc.sync.dma_start(out=at_sb[:, j, :], in_=at_d[j])
            xh_sb = pool.tile([BLK, NB + 2, BT], F32, tag="xh_sb")
            for j in range(NB + 2):
                nc.sync.dma_start(out=xh_sb[:, j, :], in_=xh_d[j])
            wes_sb = pool.tile([BLK, NB, EMB], F32, tag="wes_sb")
            for j in range(NB):
                nc.sync.dma_start(out=wes_sb[:, j, :], in_=wes_d[j])
            wl1_sb = pool.tile([BLK, 2, EMB], F32, tag="wl1_sb")
            wih_sb = pool.tile([BLK, 2, G3], F32, tag="wih_sb")
            whh_sb = pool.tile([BLK, 2, G3], F32, tag="whh_sb")
            wf0_sb = pool.tile([BLK, 2, 16], F32, tag="wf0_sb")
            for j in range(2):
                nc.sync.dma_start(out=wl1_sb[:, j, :], in_=wl1_d[j])
                nc.sync.dma_start(out=wih_sb[:, j, :], in_=wih_d[j])
                nc.sync.dma_start(out=whh_sb[:, j, :], in_=whh_d[j])
                nc.sync.dma_start(out=wf0_sb[:, j, :], in_=wf0_d[j])
            wf1_sb = pool.tile([16, 16], F32, tag="wf1_sb")
            nc.sync.dma_start(out=wf1_sb[:], in_=wf1_d[:])
            wf2_sb = pool.tile([16, 1], F32, tag="wf2_sb")
            nc.sync.dma_start(out=wf2_sb[:], in_=wf2_d[:])
            return at_sb, xh_sb, wes_sb, wl1_sb, wih_sb, whh_sb, \
                wf0_sb, wf1_sb, wf2_sb

        if not loads_in_body:
            (at_sb, xh_sb, wes_sb, wl1_sb, wih_sb, whh_sb,
             wf0_sb, wf1_sb, wf2_sb) = emit_loads(const)
        bemb_sb = const.tile([BLK, 2], F32)
        nc.sync.dma_start(out=bemb_sb[:], in_=bemb_d[:])
        bl1_sb = const.tile([BLK, 2], F32)
        nc.sync.dma_start(out=bl1_sb[:], in_=bl1_d[:])
        bxp_sb = const.tile([BLK, 6], F32)
        nc.sync.dma_start(out=bxp_sb[:], in_=bxp_d[:])
        bhn_sb = const.tile([BLK, 2], F32)
        nc.sync.dma_start(out=bhn_sb[:], in_=bhn_d[:])
        h0_sb = const.tile([BLK, 2, B], F32)
        nc.sync.dma_start(out=h0_sb[:], in_=h0_d[:])
        bf0_sb = const.tile([16, 1], F32)
        nc.sync.dma_start(out=bf0_sb[:], in_=bf0_d[:])
        bf1_sb = const.tile([16, 1], F32)
        nc.sync.dma_start(out=bf1_sb[:], in_=bf1_d[:])
        bf2_sb = const.tile([1, 1], F32)
        nc.sync.dma_start(out=bf2_sb[:], in_=bf2_d[:])

        # warm the ACT sigmoid/tanh table set while DMAs run
        dummy = const.tile([BLK, 1], F32)
        nc.vector.memset(dummy[:], 0.0)
        dummy2 = const.tile([BLK, 1], F32)
        nc.scalar.activation(dummy2[:], dummy[:], AF.Sigmoid)

        for _ in range(repeat):
            if loads_in_body:
                (at_sb, xh_sb, wes_sb, wl1_sb, wih_sb, whh_sb,
                 wf0_sb, wf1_sb, wf2_sb) = emit_loads(work)
            # ---- stage A: h1T blocks [128, 48] = A @ xfT (block tridiagonal)
            h1_sb = work.tile([BLK, NB, BT], F32)
            for i in range(NB):
                ps = psAB.tile([BLK, BT], F32, tag="ps")
                for jo in range(3):
                    nc.tensor.matmul(
                        ps[:], at_sb[:, 3 * i + jo, :], xh_sb[:, i + jo, :],
                        start=(jo == 0), stop=(jo == 2))
                if i % 2 == 0:
                    nc.vector.tensor_copy(h1_sb[:, i, :], ps[:])
                else:
                    nc.scalar.activation(h1_sb[:, i, :], ps[:], AF.Identity)

            # ---- stage B: partial t2preT [256, 48] = W_es @ h1
            t2p_sb = work.tile([BLK, 2, BT], F32)
            for e in range(2):
                ps = psAB.tile([BLK, BT], F32, tag="ps")
                for i in range(NB):
                    nc.tensor.matmul(
                        ps[:], wes_sb[:, i, e * BLK:(e + 1) * BLK],
                        h1_sb[:, i, :], start=(i == 0), stop=(i == NB - 1))
                nc.vector.tensor_copy(t2p_sb[:, e, :], ps[:])

            # ---- AllReduce of t2 pre-activations over the 8 cores
            cc_in = dram.tile([2, BLK, BT], F32)
            cc_out = dram.tile([2, BLK, BT], F32)
            for e in range(2):
                nc.gpsimd.dma_start(out=cc_in[e], in_=t2p_sb[:, e, :])
            nc.gpsimd.collective_compute(
                "AllReduce", ALU.add,
                replica_groups=[list(range(N_CORES))],
                ins=[cc_in.opt()], outs=[cc_out.opt()])
            t2r_sb = work.tile([BLK, 2, BT], F32)
            for e in range(2):
                nc.gpsimd.dma_start(out=t2r_sb[:, e, :], in_=cc_out[e])

            # sigmoid(t2pre + b_emb)
            t2_sb = work.tile([BLK, 2, BT], F32)
            for e in range(2):
                nc.scalar.activation(t2_sb[:, e, :], t2r_sb[:, e, :],
                                     AF.Sigmoid, bias=bemb_sb[:, e:e + 1])

            # ---- stage C: t4T = sigmoid(W_l1 @ t2T + b_l1)
            t4_sb = work.tile([BLK, 2, BT], F32)
            for mc in range(2):
                ps = psAB.tile([BLK, BT], F32, tag="ps")
                for kc in range(2):
                    nc.tensor.matmul(
                        ps[:], wl1_sb[:, kc, mc * BLK:(mc + 1) * BLK],
                        t2_sb[:, kc, :], start=(kc == 0), stop=(kc == 1))
                nc.scalar.activation(t4_sb[:, mc, :], ps[:], AF.Sigmoid,
                                     bias=bl1_sb[:, mc:mc + 1])

            # ---- stage D: xpT [128, 6, 4, 12] = W_ih @ t4T (+ gate biases)
            ps_xp = psX.tile([BLK, 6, B, T], F32)
            for c in range(6):
                for kc in range(2):
                    nc.tensor.matmul(
                        ps_xp[:, c, :, :], wih_sb[:, kc, c * BLK:(c + 1) * BLK],
                        t4_sb[:, kc, :], start=(kc == 0), stop=(kc == 1))
            xp_sb = work.tile([BLK, 6, B, T], F32)
            for c in range(6):
                if c % 2 == 0:
                    nc.vector.tensor_scalar_add(
                        xp_sb[:, c, :, :], ps_xp[:, c, :, :], bxp_sb[:, c:c + 1])
                else:
                    nc.scalar.activation(
                        xp_sb[:, c, :, :], ps_xp[:, c, :, :], AF.Identity,
                        bias=bxp_sb[:, c:c + 1])

            # ---- GRU over T steps, h tile [128, 2, 4]
            h_prev = h0_sb
            for t in range(T):
                ps_g = psG.tile([BLK, 6, B], F32)
                for c in range(6):
                    for kc in range(2):
                        nc.tensor.matmul(
                            ps_g[:, c, :],
                            whh_sb[:, kc, c * BLK:(c + 1) * BLK],
                            h_prev[:, kc, :], start=(kc == 0), stop=(kc == 1))
                # r,z pre-activations + sigmoid
                rzp = gru.tile([BLK, 4, B], F32, tag="rzp")
                nc.vector.tensor_add(rzp[:], ps_g[:, 0:4, :], xp_sb[:, 0:4, :, t])
                rz = gru.tile([BLK, 4, B], F32, tag="rz")
                nc.scalar.activation(rz[:], rzp[:], AF.Sigmoid)
                # n gate: nw = tanh(xn + r * (hn + b_hh_n))
                npre = gru.tile([BLK, 2, B], F32, tag="npre")
                for cc in range(2):
                    nc.vector.scalar_tensor_tensor(
                        npre[:, cc, :], ps_g[:, 4 + cc, :],
                        bhn_sb[:, cc:cc + 1], rz[:, cc, :],
                        op0=ALU.add, op1=ALU.mult)
                nin = gru.tile([BLK, 2, B], F32, tag="nin")
                nc.vector.tensor_add(nin[:], npre[:], xp_sb[:, 4:6, :, t])
                nw = gru.tile([BLK, 2, B], F32, tag="nw")
                nc.scalar.activation(nw[:], nin[:], AF.Tanh)
                # h' = (1-z)*nw + z*h  (u = z*h and v = 1-z off critical path)
                u = gru.tile([BLK, 2, B], F32, tag="u")
                nc.vector.tensor_mul(u[:], rz[:, 2:4, :], h_prev[:])
                v = gru.tile([BLK, 2, B], F32, tag="v")
                nc.vector.tensor_scalar(v[:], rz[:, 2:4, :], -1.0, 1.0,
                                        op0=ALU.mult, op1=ALU.add)
                w = gru.tile([BLK, 2, B], F32, tag="w")
                nc.vector.tensor_mul(w[:], nw[:], v[:])
                h_new = gru.tile([BLK, 2, B], F32, tag="h")
                nc.vector.tensor_add(h_new[:], w[:], u[:])
                h_prev = h_new

            # ---- tail MLP: [4,256] -> 16 -> 16 -> 1, sigmoid each
            ps_o1 = psG.tile([16, B], F32, tag="o")
            for kc in range(2):
                nc.tensor.matmul(ps_o1[:], wf0_sb[:, kc, :], h_prev[:, kc, :],
                                 start=(kc == 0), stop=(kc == 1))
            o1 = work.tile([16, B], F32, tag="o1s")
            nc.scalar.activation(o1[:], ps_o1[:], AF.Sigmoid, bias=bf0_sb[:])
            ps_o2 = psG.tile([16, B], F32, tag="o")
            nc.tensor.matmul(ps_o2[:], wf1_sb[:], o1[:], start=True, stop=True)
            o2 = work.tile([16, B], F32, tag="o2s")
            nc.scalar.activation(o2[:], ps_o2[:], AF.Sigmoid, bias=bf1_sb[:])
            ps_o3 = psG.tile([1, B], F32, tag="o")
            nc.tensor.matmul(ps_o3[:], wf2_sb[:], o2[:], start=True, stop=True)
            o3 = work.tile([1, B], F32, tag="o3s")
            nc.scalar.activation(o3[:], ps_o3[:], AF.Sigmoid, bias=bf2_sb[:])
            nc.sync.dma_start(out=out_d[:], in_=o3[:])

    nc.compile()
    return nc


# ---------------------------------------------------------------- host side

def prepare_in_maps(x, h0, rows, cols, W_emb, b_emb, W_l1, b_l1,
                    W_ih, W_hh, b_ih, b_hh, W_f0, b_f0, W_f1, b_f1,
                    W_f2, b_f2):
    f32 = np.float32
    x = np.ascontiguousarray(x, f32)
    assert int(rows.max()) < SUP and int(cols.max()) < SUP

    # dense banded adjacency on its true support (duplicates sum = coalesce)
    A = np.zeros((SUP, SUP), f32)
    np.add.at(A, (np.asarray(rows), np.asarray(cols)), 1.0)

    S_pad = NBLOCKS * BLK
    ATp = np.zeros((S_pad, S_pad), f32)
    ATp[:SUP, :SUP] = A.T

    xf = x.reshape(BT, N)
    # xsT padded with one leading zero block (halo for core 0) + tail blocks
    XTp = np.zeros(((NBLOCKS + 2) * BLK, BT), f32)
    XTp[BLK:BLK + SUP] = xf[:, :SUP].T

    WesT = np.zeros((S_pad, EMB), f32)
    WesT[:SUP] = np.asarray(W_emb, f32)[:, :SUP].T

    def pm(vec, k):  # partition-major [128, k] view of a length 128*k vector
        return np.ascontiguousarray(
            np.asarray(vec, f32).reshape(k, BLK).T)

    bih = np.asarray(b_ih, f32)
    bhh = np.asarray(b_hh, f32)
    bxp = np.concatenate([bih[:512] + bhh[:512], bih[512:]])  # rz: both, n: ih
    h0c = np.ascontiguousarray(
        np.asarray(h0, f32)[0].T.reshape(2, BLK, B).transpose(1, 0, 2)
    ).reshape(BLK, 8)

    common = dict(
        wl1t=np.ascontiguousarray(np.asarray(W_l1, f32).T.reshape(2, BLK, EMB)),
        wiht=np.ascontiguousarray(np.asarray(W_ih, f32).T.reshape(2, BLK, G3)),
        whht=np.ascontiguousarray(np.asarray(W_hh, f32).T.reshape(2, BLK, G3)),
        wf0t=np.ascontiguousarray(np.asarray(W_f0, f32).T.reshape(2, BLK, 16)),
        wf1t=np.ascontiguousarray(np.asarray(W_f1, f32).T),
        wf2t=np.ascontiguousarray(np.asarray(W_f2, f32).T),
        bemb=pm(b_emb, 2), bl1=pm(b_l1, 2),
        bxp=pm(bxp, 6), bhn=pm(bhh[512:], 2),
        h0c=h0c,
        bf0=np.asarray(b_f0, f32).reshape(16, 1),
        bf1=np.asarray(b_f1, f32).reshape(16, 1),
        bf2=np.asarray(b_f2, f32).reshape(1, 1),
    )

    in_maps = []
    for c in range(N_CORES):
        at = np.zeros((3 * NB, BLK, BLK), f32)
        for i in range(NB):
            I = NB * c + i
            for jo in range(3):
                J = I - 1 + jo
                if 0 <= J < NBLOCKS:
                    at[3 * i + jo] = ATp[J * BLK:(J + 1) * BLK,
                                         I * BLK:(I + 1) * BLK]
        xh = np.ascontiguousarray(
            XTp[NB * c * BLK:(NB * c + NB + 2) * BLK]).reshape(NB + 2, BLK, BT)
        wes = np.ascontiguousarray(
            WesT[NB * c * BLK:(NB * (c + 1)) * BLK]).reshape(NB, BLK, EMB)
        in_maps.append(dict(at=at, xh=xh, wes=wes, **common))
        in_maps[-1].update(common)
    return in_maps


_CACHE = {}


def kernel(**inputs) -> np.ndarray:
    if "nc" not in _CACHE:
        _CACHE["nc"] = build_program()
    nc = _CACHE["nc"]
    in_maps = prepare_in_maps(**inputs)
    res = run_bass_kernel_spmd(nc, in_maps, list(range(N_CORES)))
    out = res.results[0]["out"]          # [1, 4]
    return np.ascontiguousarray(out.T.astype(np.float32))  # [4, 1]


if __name__ == "__main__":
    import importlib.util
    spec = importlib.util.spec_from_file_location("reference", "reference.py")
    ref = importlib.util.module_from_spec(spec)
    spec.loader.exec_module(ref)
    inputs = {k: np.asarray(v) for k, v in ref.setup_inputs().items()}
    expected = np.asarray(ref.reference(**inputs))
    got = kernel(**inputs)
    err = np.abs(got - expected).max() / np.abs(expected).max()
    print("expected:", expected.ravel())
    print("got:     ", got.ravel())
    print("Relative error:", err)


# revision 13
# speedup vs baseline: 304.2259x; 304.2259x over previous
"""Bass/Trainium2 kernel for nn_HailNet_42975442763785 (GNN message passing).

Math insight: the COO adjacency built by the model only references node
indices in [0, 4111) (kj = k + j with k<16, j<4096; all other index vectors
stay below 4111), and the coalesced matrix A is banded with offsets in
[-80, 80].  Therefore h1 = xf @ A.T is zero outside its first 4111 columns
and the [48,65536] @ [65536,256] embedding matmul reduces exactly to
[48,4111] @ [4111,256].  Stage A (A @ xfT) becomes a block-tridiagonal
matmul over 128-wide blocks.

Sharding (8 cores):
  - The 4111 (padded to 5120 = 40*128) contraction axis is split 5 blocks
    per core.  Stage A needs only a 1-block halo of xfT -> no communication.
  - Stage B computes per-core partial t2 pre-activations [256, 48];
    one AllReduce combines them.
  - The tiny tail (lin1, GRU over T=12, final MLP) is computed redundantly
    on every core; core 0's output is returned.

Everything on-device runs in a feature-on-partition layout ("T layout"):
activations are [128, ...] tiles with the feature dim on partitions and
(batch*time) on the free dim, so all elementwise GRU ops are [128, <=16].
"""

from contextlib import ExitStack

import numpy as np

import concourse.bass as bass
import concourse.tile as tile
from concourse import bacc, mybir
from concourse.bass_utils import run_bass_kernel_spmd

F32 = mybir.dt.float32
AF = mybir.ActivationFunctionType
ALU = mybir.AluOpType

N_CORES = 8
BLK = 128
NB = 5                    # I-blocks per core
NBLOCKS = N_CORES * NB    # 40 padded blocks
SUP = 4111                # true support of the adjacency
N = 65536
BT, B, T = 48, 4, 12
EMB, HID, G3 = 256, 256, 768


# ---------------------------------------------------------------- device code

def build_program(repeat: int = 1, loads_in_body: bool = False, use_collective: bool = True,
                  replicated: bool = False, ab_bf16: bool = False,
                  gru_bf16: bool = False):
    """loads_in_body=True re-DMAs all large inputs every repeat iteration so
    the repeat-slope timing includes the input-streaming cost of a one-shot
    run (this is a memory-regime problem)."""
    nc = bacc.Bacc("TRN2", target_bir_lowering=False, debug=False,
                   num_devices=N_CORES)

    # per-core sharded (or fully replicated) inputs, partition-major [128, F]
    nb = 33 if replicated else NB        # I-blocks handled by this core
    AB = mybir.dt.bfloat16 if ab_bf16 else F32
    GD = mybir.dt.bfloat16 if gru_bf16 else F32
    at_d = nc.dram_tensor("at", [BLK, 3 * nb, BLK], AB, kind="ExternalInput")
    xh_d = nc.dram_tensor("xh", [BLK, nb + 2, BT], AB, kind="ExternalInput")
    wes_d = nc.dram_tensor("wes", [BLK, nb, EMB], AB, kind="ExternalInput")
    # replicated weights
    wl1_d = nc.dram_tensor("wl1t", [BLK, 2, EMB], F32, kind="ExternalInput")
    wih_d = nc.dram_tensor("wiht", [BLK, 2, G3], F32, kind="ExternalInput")
    whh_d = nc.dram_tensor("whht", [BLK, 2, G3], GD, kind="ExternalInput")
    wf0_d = nc.dram_tensor("wf0t", [BLK, 2, 16], GD, kind="ExternalInput")
    wf1_d = nc.dram_tensor("wf1t", [16, 16], F32, kind="ExternalInput")
    wf2_d = nc.dram_tensor("wf2t", [16, 1], F32, kind="ExternalInput")
    bemb_d = nc.dram_tensor("bemb", [BLK, 2], F32, kind="ExternalInput")
    bl1_d = nc.dram_tensor("bl1", [BLK, 2], F32, kind="ExternalInput")
    bxp_d = nc.dram_tensor("bxp", [BLK, 6], F32, kind="ExternalInput")
    bhn_d = nc.dram_tensor("bhn", [BLK, 2], F32, kind="ExternalInput")
    h0_d = nc.dram_tensor("h0c", [BLK, 8], GD, kind="ExternalInput")
    bf0_d = nc.dram_tensor("bf0", [16, 1], F32, kind="ExternalInput")
    bf1_d = nc.dram_tensor("bf1", [16, 1], F32, kind="ExternalInput")
    bf2_d = nc.dram_tensor("bf2", [1, 1], F32, kind="ExternalInput")
    out_d = nc.dram_tensor("out", [1, B], F32, kind="ExternalOutput")

    with tile.TileContext(nc) as tc, ExitStack() as ctx:
        const = ctx.enter_context(tc.tile_pool(name="const", bufs=1))
        work = ctx.enter_context(tc.tile_pool(name="work", bufs=2))
        gru = ctx.enter_context(tc.tile_pool(name="gru", bufs=2))
        psAB = ctx.enter_context(tc.tile_pool(name="psAB", bufs=2, space="PSUM"))
        psX = ctx.enter_context(tc.tile_pool(name="psX", bufs=1, space="PSUM"))
        psG = ctx.enter_context(tc.tile_pool(name="psG", bufs=2, space="PSUM"))
        dram = ctx.enter_context(tc.tile_pool(name="dram", bufs=2, space="DRAM"))

        def emit_loads(pool):
            """DMA the large per-core inputs + weights into SBUF tiles."""
            at_sb = pool.tile([BLK, 3 * nb, BLK], AB, tag="at_sb")
            nc.sync.dma_start(out=at_sb[:], in_=at_d[:])
            xh_sb = pool.tile([BLK, nb + 2, BT], AB, tag="xh_sb")
            nc.sync.dma_start(out=xh_sb[:], in_=xh_d[:])
            wes_sb = pool.tile([BLK, nb, EMB], AB, tag="wes_sb")
            nc.sync.dma_start(out=wes_sb[:], in_=wes_d[:])
            wl1_sb = pool.tile([BLK, 2, EMB], F32, tag="wl1_sb")
            nc.sync.dma_start(out=wl1_sb[:], in_=wl1_d[:])
            wih_sb = pool.tile([BLK, 2, G3], F32, tag="wih_sb")
            nc.sync.dma_start(out=wih_sb[:], in_=wih_d[:])
            whh_sb = pool.tile([BLK, 2, G3], GD, tag="whh_sb")
            nc.sync.dma_start(out=whh_sb[:], in_=whh_d[:])
            wf0_sb = pool.tile([BLK, 2, 16], GD, tag="wf0_sb")
            nc.sync.dma_start(out=wf0_sb[:], in_=wf0_d[:])
            wf1_sb = pool.tile([16, 16], F32, tag="wf1_sb")
            nc.sync.dma_start(out=wf1_sb[:], in_=wf1_d[:])
            wf2_sb = pool.tile([16, 1], F32, tag="wf2_sb")
            nc.sync.dma_start(out=wf2_sb[:], in_=wf2_d[:])
            return at_sb, xh_sb, wes_sb, wl1_sb, wih_sb, whh_sb, \
                wf0_sb, wf1_sb, wf2_sb

        if not loads_in_body:
            (at_sb, xh_sb, wes_sb, wl1_sb, wih_sb, whh_sb,
             wf0_sb, wf1_sb, wf2_sb) = emit_loads(const)
        bemb_sb = const.tile([BLK, 2], F32)
        nc.sync.dma_start(out=bemb_sb[:], in_=bemb_d[:])
        bl1_sb = const.tile([BLK, 2], F32)
        nc.sync.dma_start(out=bl1_sb[:], in_=bl1_d[:])
        bxp_sb = const.tile([BLK, 6], F32)
        nc.sync.dma_start(out=bxp_sb[:], in_=bxp_d[:])
        bhn_sb = const.tile([BLK, 2], F32)
        nc.sync.dma_start(out=bhn_sb[:], in_=bhn_d[:])
        h0_sb = const.tile([BLK, 2, B], GD)
        nc.sync.dma_start(out=h0_sb[:], in_=h0_d[:])
        bf0_sb = const.tile([16, 1], F32)
        nc.sync.dma_start(out=bf0_sb[:], in_=bf0_d[:])
        bf1_sb = const.tile([16, 1], F32)
        nc.sync.dma_start(out=bf1_sb[:], in_=bf1_d[:])
        bf2_sb = const.tile([1, 1], F32)
        nc.sync.dma_start(out=bf2_sb[:], in_=bf2_d[:])

        # warm the ACT sigmoid/tanh table set while DMAs run
        dummy = const.tile([BLK, 1], F32)
        nc.vector.memset(dummy[:], 0.0)
        dummy2 = const.tile([BLK, 1], F32)
        nc.scalar.activation(dummy2[:], dummy[:], AF.Sigmoid)

        for _ in range(repeat):
            if loads_in_body:
                (at_sb, xh_sb, wes_sb, wl1_sb, wih_sb, whh_sb,
                 wf0_sb, wf1_sb, wf2_sb) = emit_loads(work)
            # ---- stage A: h1T blocks [128, 48] = A @ xfT (block tridiagonal)
            h1_sb = work.tile([BLK, nb, BT], AB)
            for i in range(nb):
                ps = psAB.tile([BLK, BT], F32, tag="ps")
                for jo in range(3):
                    nc.tensor.matmul(
                        ps[:], at_sb[:, 3 * i + jo, :], xh_sb[:, i + jo, :],
                        start=(jo == 0), stop=(jo == 2))
                if i % 2 == 0:
                    nc.vector.tensor_copy(h1_sb[:, i, :], ps[:])
                else:
                    nc.scalar.activation(h1_sb[:, i, :], ps[:], AF.Identity)

            # ---- stage B: partial t2preT [256, 48] = W_es @ h1
            t2p_sb = work.tile([BLK, 2, BT], F32)
            for e in range(2):
                ps = psAB.tile([BLK, BT], F32, tag="ps")
                for i in range(nb):
                    nc.tensor.matmul(
                        ps[:], wes_sb[:, i, e * BLK:(e + 1) * BLK],
                        h1_sb[:, i, :], start=(i == 0), stop=(i == nb - 1))
                nc.vector.tensor_copy(t2p_sb[:, e, :], ps[:])

            # ---- AllReduce of t2 pre-activations over the 8 cores
            cc_in = dram.tile([2, BLK, BT], F32)
            cc_out = dram.tile([2, BLK, BT], F32)
            for e in range(2):
                nc.gpsimd.dma_start(out=cc_in[e], in_=t2p_sb[:, e, :])
            if use_collective and not replicated:
                nc.gpsimd.collective_compute(
                    "AllReduce", ALU.add,
                    replica_groups=[list(range(N_CORES))],
                    ins=[cc_in.opt()], outs=[cc_out.opt()])
            else:
                nc.gpsimd.dma_start(out=cc_out[:], in_=cc_in[:])
            t2r_sb = work.tile([BLK, 2, BT], F32)
            for e in range(2):
                nc.gpsimd.dma_start(out=t2r_sb[:, e, :], in_=cc_out[e])

            # sigmoid(t2pre + b_emb)
            t2_sb = work.tile([BLK, 2, BT], F32)
            for e in range(2):
                nc.scalar.activation(t2_sb[:, e, :], t2r_sb[:, e, :],
                                     AF.Sigmoid, bias=bemb_sb[:, e:e + 1])

            # ---- stage C: t4T = sigmoid(W_l1 @ t2T + b_l1)
            t4_sb = work.tile([BLK, 2, BT], F32)
            for mc in range(2):
                ps = psAB.tile([BLK, BT], F32, tag="ps")
                for kc in range(2):
                    nc.tensor.matmul(
                        ps[:], wl1_sb[:, kc, mc * BLK:(mc + 1) * BLK],
                        t2_sb[:, kc, :], start=(kc == 0), stop=(kc == 1))
                nc.scalar.activation(t4_sb[:, mc, :], ps[:], AF.Sigmoid,
                                     bias=bl1_sb[:, mc:mc + 1])

            # ---- stage D: xpT [128, 6, 4, 12] = W_ih @ t4T (+ gate biases)
            ps_xp = psX.tile([BLK, 6, B, T], F32)
            for c in range(6):
                for kc in range(2):
                    nc.tensor.matmul(
                        ps_xp[:, c, :, :], wih_sb[:, kc, c * BLK:(c + 1) * BLK],
                        t4_sb[:, kc, :], start=(kc == 0), stop=(kc == 1))
            xp_sb = work.tile([BLK, 6, B, T], F32)
            for c in range(6):
                if c % 2 == 0:
                    nc.vector.tensor_scalar_add(
                        xp_sb[:, c, :, :], ps_xp[:, c, :, :], bxp_sb[:, c:c + 1])
                else:
                    nc.scalar.activation(
                        xp_sb[:, c, :, :], ps_xp[:, c, :, :], AF.Identity,
                        bias=bxp_sb[:, c:c + 1])

            # ---- GRU over T steps, h tile [128, 2, 4]
            h_prev = h0_sb
            for t in range(T):
                ps_g = psG.tile([BLK, 6, B], F32)
                for c in range(6):
                    for kc in range(2):
                        nc.tensor.matmul(
                            ps_g[:, c, :],
                            whh_sb[:, kc, c * BLK:(c + 1) * BLK],
                            h_prev[:, kc, :], start=(kc == 0), stop=(kc == 1))
                # r,z pre-activations + sigmoid
                rzp = gru.tile([BLK, 4, B], F32, tag="rzp")
                nc.vector.tensor_add(rzp[:], ps_g[:, 0:4, :], xp_sb[:, 0:4, :, t])
                rz = gru.tile([BLK, 4, B], F32, tag="rz")
                nc.scalar.activation(rz[:], rzp[:], AF.Sigmoid)
                # n gate: nw = tanh(xn + r * (hn + b_hh_n))
                npre = gru.tile([BLK, 2, B], F32, tag="npre")
                for cc in range(2):
                    nc.vector.scalar_tensor_tensor(
                        npre[:, cc, :], ps_g[:, 4 + cc, :],
                        bhn_sb[:, cc:cc + 1], rz[:, cc, :],
                        op0=ALU.add, op1=ALU.mult)
                nin = gru.tile([BLK, 2, B], F32, tag="nin")
                nc.vector.tensor_add(nin[:], npre[:], xp_sb[:, 4:6, :, t])
                nw = gru.tile([BLK, 2, B], F32, tag="nw")
                nc.scalar.activation(nw[:], nin[:], AF.Tanh)
                # h' = (1-z)*nw + z*h  (u = z*h and v = 1-z off critical path)
                u = gru.tile([BLK, 2, B], F32, tag="u")
                nc.vector.tensor_mul(u[:], rz[:, 2:4, :], h_prev[:])
                v = gru.tile([BLK, 2, B], F32, tag="v")
                nc.vector.tensor_scalar(v[:], rz[:, 2:4, :], -1.0, 1.0,
                                        op0=ALU.mult, op1=ALU.add)
                w = gru.tile([BLK, 2, B], F32, tag="w")
                nc.vector.tensor_mul(w[:], nw[:], v[:])
                h_new = gru.tile([BLK, 2, B], GD, tag="h")
                nc.vector.tensor_add(h_new[:], w[:], u[:])
                h_prev = h_new

            # ---- tail MLP: [4,256] -> 16 -> 16 -> 1, sigmoid each
            ps_o1 = psG.tile([16, B], F32, tag="o")
            for kc in range(2):
                nc.tensor.matmul(ps_o1[:], wf0_sb[:, kc, :], h_prev[:, kc, :],
                                 start=(kc == 0), stop=(kc == 1))
            o1 = work.tile([16, B], F32, tag="o1s")
            nc.scalar.activation(o1[:], ps_o1[:], AF.Sigmoid, bias=bf0_sb[:])
            ps_o2 = psG.tile([16, B], F32, tag="o")
            nc.tensor.matmul(ps_o2[:], wf1_sb[:], o1[:], start=True, stop=True)
            o2 = work.tile([16, B], F32, tag="o2s")
            nc.scalar.activation(o2[:], ps_o2[:], AF.Sigmoid, bias=bf1_sb[:])
            ps_o3 = psG.tile([1, B], F32, tag="o")
            nc.tensor.matmul(ps_o3[:], wf2_sb[:], o2[:], start=True, stop=True)
            o3 = work.tile([1, B], F32, tag="o3s")
            nc.scalar.activation(o3[:], ps_o3[:], AF.Sigmoid, bias=bf2_sb[:])
            nc.sync.dma_start(out=out_d[:], in_=o3[:])

    nc.compile()
    return nc


# ---------------------------------------------------------------- host side

def prepare_in_maps(x, h0, rows, cols, W_emb, b_emb, W_l1, b_l1,
                    W_ih, W_hh, b_ih, b_hh, W_f0, b_f0, W_f1, b_f1,
                    W_f2, b_f2, replicated=False, ab_bf16=False,
                    gru_bf16=False):
    import ml_dtypes
    f32 = np.float32
    abt = ml_dtypes.bfloat16 if ab_bf16 else f32
    gdt = ml_dtypes.bfloat16 if gru_bf16 else f32
    x = np.ascontiguousarray(x, f32)
    assert int(rows.max()) < SUP and int(cols.max()) < SUP

    # dense banded adjacency on its true support (duplicates sum = coalesce)
    A = np.zeros((SUP, SUP), f32)
    np.add.at(A, (np.asarray(rows), np.asarray(cols)), 1.0)

    S_pad = NBLOCKS * BLK
    ATp = np.zeros((S_pad, S_pad), f32)
    ATp[:SUP, :SUP] = A.T

    xf = x.reshape(BT, N)
    # xsT padded with one leading zero block (halo for core 0) + tail blocks
    XTp = np.zeros(((NBLOCKS + 2) * BLK, BT), f32)
    XTp[BLK:BLK + SUP] = xf[:, :SUP].T

    WesT = np.zeros((S_pad, EMB), f32)
    WesT[:SUP] = np.asarray(W_emb, f32)[:, :SUP].T

    def pm(vec, k):  # partition-major [128, k] view of a length 128*k vector
        return np.ascontiguousarray(
            np.asarray(vec, f32).reshape(k, BLK).T)

    bih = np.asarray(b_ih, f32)
    bhh = np.asarray(b_hh, f32)
    bxp = np.concatenate([bih[:512] + bhh[:512], bih[512:]])  # rz: both, n: ih
    h0c = np.ascontiguousarray(
        np.asarray(h0, f32)[0].T.reshape(2, BLK, B).transpose(1, 0, 2)
    ).reshape(BLK, 8)

    def pm3(w, k):  # [K, M] weight -> partition-major [128, K//128, M]
        return np.ascontiguousarray(
            np.asarray(w, f32).T.reshape(-1, BLK, k).transpose(1, 0, 2))

    common = dict(
        wl1t=pm3(W_l1, EMB),
        wiht=pm3(W_ih, G3),
        whht=pm3(W_hh, G3).astype(gdt),
        wf0t=pm3(W_f0, 16).astype(gdt),
        wf1t=np.ascontiguousarray(np.asarray(W_f1, f32).T),
        wf2t=np.ascontiguousarray(np.asarray(W_f2, f32).T),
        bemb=pm(b_emb, 2), bl1=pm(b_l1, 2),
        bxp=pm(bxp, 6), bhn=pm(bhh[512:], 2),
        h0c=h0c.astype(gdt),
        bf0=np.asarray(b_f0, f32).reshape(16, 1),
        bf1=np.asarray(b_f1, f32).reshape(16, 1),
        bf2=np.asarray(b_f2, f32).reshape(1, 1),
    )

    if replicated:
        nb = 33
        at = np.zeros((3 * nb, BLK, BLK), f32)
        for i in range(nb):
            for jo in range(3):
                J = i - 1 + jo
                if 0 <= J < nb:
                    at[3 * i + jo] = ATp[J * BLK:(J + 1) * BLK,
                                         i * BLK:(i + 1) * BLK]
        xh = np.ascontiguousarray(
            XTp[:(nb + 2) * BLK].reshape(nb + 2, BLK, BT).transpose(1, 0, 2))
        wes = np.ascontiguousarray(
            WesT[:nb * BLK].reshape(nb, BLK, EMB).transpose(1, 0, 2))
        m = dict(at=np.ascontiguousarray(at.transpose(1, 0, 2)).astype(abt),
                 xh=xh.astype(abt), wes=wes.astype(abt), **common)
        return [m] * N_CORES

    in_maps = []
    for c in range(N_CORES):
        at = np.zeros((3 * NB, BLK, BLK), f32)
        for i in range(NB):
            I = NB * c + i
            for jo in range(3):
                J = I - 1 + jo
                if 0 <= J < NBLOCKS:
                    at[3 * i + jo] = ATp[J * BLK:(J + 1) * BLK,
                                         I * BLK:(I + 1) * BLK]
        xh = np.ascontiguousarray(
            XTp[NB * c * BLK:(NB * c + NB + 2) * BLK]
            .reshape(NB + 2, BLK, BT).transpose(1, 0, 2))
        wes = np.ascontiguousarray(
            WesT[NB * c * BLK:(NB * (c + 1)) * BLK]
            .reshape(NB, BLK, EMB).transpose(1, 0, 2))
        in_maps.append(dict(at=np.ascontiguousarray(at.transpose(1, 0, 2)).astype(abt),
                            xh=xh.astype(abt), wes=wes.astype(abt), **common))
    return in_maps


_CACHE = {}


def kernel(**inputs) -> np.ndarray:
    if "nc" not in _CACHE:
        _CACHE["nc"] = build_program()
    nc = _CACHE["nc"]
    in_maps = prepare_in_maps(**inputs)
    res = run_bass_kernel_spmd(nc, in_maps, list(range(N_CORES)))
    out = res.results[0]["out"]          # [1, 4]
    return np.ascontiguousarray(out.T.astype(np.float32))  # [4, 1]


if __name__ == "__main__":
    import importlib.util
    spec = importlib.util.spec_from_file_location("reference", "reference.py")
    ref = importlib.util.module_from_spec(spec)
    spec.loader.exec_module(ref)
    inputs = {k: np.asarray(v) for k, v in ref.setup_inputs().items()}
    expected = np.asarray(ref.reference(**inputs))
    got = kernel(**inputs)
    err = np.abs(got - expected).max() / np.abs(expected).max()
    print("expected:", expected.ravel())
    print("got:     ", got.ravel())
    print("Relative error:", err)
